# revision 25
# baseline (speedup 1.0000x reference)
"""CorrCosine TRN2 kernel.

out[b, i, j, h, w] = <cur[b,:,i,j]/||cur[b,:,i,j]||, ref[b,:,h,w]/||ref[b,:,h,w]||>

Data-parallel over batch B=8 across the 8 NeuronCores; per core one
[4096 x 256] @ [256 x 4096] GEMM plus the two L2 normalizations.

The original fp32 version was output-DMA-bound (64 MiB fp32 result
~190us of HBM writes per core).  This version runs the whole pipeline in
fp16: inputs are cast on the host, the GEMM runs fp16 x fp16 -> fp32
PSUM (full PE rate + fast weight load), the output is stored fp16 and
upcast on the host.  Norms are computed on-chip in fp32 via an all-ones
stationary matmul (which leaves the result broadcast across all 128
partitions), and BOTH operands are pre-scaled by 1/norm on the
otherwise-idle GpSimd engine, so PSUM evacuation is a plain copy split
between ACT and DVE.  All output DMA goes through the sync-engine HWDGE
ring, one 512 KiB transfer per half m-tile, so the ACT/DVE queues never
stall on each other's completions.
"""

import numpy as np

from concourse import bacc, mybir
from concourse import tile
from concourse.bass_utils import run_bass_kernel_spmd

B, C, H, W = 8, 256, 64, 64
HW = H * W            # 4096
P = 128               # partitions
KT = C // P           # 2 k-tiles
FD = 512              # psum bank free dim (fp32)
NCH = HW // FD        # 8 norm chunks
MT = HW // P          # 32 m-tiles
IBW = 2048            # input DMA width (512 KiB fp16 chunks)

f16 = mybir.dt.float16
f32 = mybir.dt.float32
AF = mybir.ActivationFunctionType

_cached_nc = None


def _build():
    nc = bacc.Bacc("TRN2", target_bir_lowering=False, debug=False)
    cur_d = nc.dram_tensor("cur", [C, HW], f16, kind="ExternalInput")
    ref_d = nc.dram_tensor("ref", [C, HW], f16, kind="ExternalInput")
    out_d = nc.dram_tensor("out", [HW, HW], f16, kind="ExternalOutput")

    with tile.TileContext(nc) as tc:
        with (
            tc.tile_pool(name="opnd", bufs=1) as opnd,
            tc.tile_pool(name="cst", bufs=1) as cstp,
            tc.tile_pool(name="ps", bufs=8, space="PSUM") as psp,
        ):
            ones_f = cstp.tile([P, P], f32, tag="ones_f", name="ones_f")
            nc.gpsimd.memset(ones_f[:], 1.0)
            ones = cstp.tile([P, P], f16, tag="ones", name="ones")
            nc.vector.tensor_copy(ones[:], ones_f[:])

            raw = {}
            scl = {}
            for t in ("ref", "cur"):
                for k in range(KT):
                    raw[t, k] = opnd.tile(
                        [P, HW], f16, tag=f"raw_{t}{k}", name=f"raw_{t}{k}"
                    )
                    scl[t, k] = opnd.tile(
                        [P, HW], f16, tag=f"scl_{t}{k}", name=f"scl_{t}{k}"
                    )

            # halves-first input order: ref h0 -> cur h0 -> ref h1 -> cur h1,
            # so ref normalization can start after two 512 KiB DMAs.
            # HWDGE (sync ring) for the ~0.6us first-byte latency.
            for i in range(HW // IBW):
                for t in ("ref", "cur"):
                    src = ref_d if t == "ref" else cur_d
                    for k in range(KT):
                        nc.sync.dma_start(
                            raw[t, k][:, i * IBW:(i + 1) * IBW],
                            src[k * P:(k + 1) * P, i * IBW:(i + 1) * IBW],
                        )

            with (
                tc.tile_pool(name="sq", bufs=2) as sqp,
                tc.tile_pool(name="nrm", bufs=2) as nrmp,
            ):
                def norm_group(t, g):
                    # 1/||.|| for columns [g*2048, (g+1)*2048) of tensor t,
                    # broadcast on all partitions; scale both k-tiles with
                    # it.  One wide op per engine pass amortizes the fixed
                    # per-instruction latency; fp16 squares keep the
                    # ones-matmul on the fast PE path.
                    gl = slice(g * IBW, (g + 1) * IBW)
                    sq0 = sqp.tile([P, IBW], f16, tag="sq", name=f"sq0_{t}{g}")
                    sq1 = sqp.tile([P, IBW], f16, tag="sq", name=f"sq1_{t}{g}")
                    nc.scalar.activation(sq0[:], raw[t, 0][:, gl], AF.Square)
                    nc.scalar.activation(sq1[:], raw[t, 1][:, gl], AF.Square)
                    nrm = nrmp.tile([P, IBW], f32, tag="nrm", name=f"nrm_{t}{g}")
                    for c in range(IBW // FD):
                        cl = slice(c * FD, (c + 1) * FD)
                        ss = psp.tile(
                            [P, FD], f32, tag="ss", name=f"ss_{t}{g}{c}", bufs=2
                        )
                        nc.tensor.matmul(ss[:], ones[:], sq0[:, cl], start=True, stop=False)
                        nc.tensor.matmul(ss[:], ones[:], sq1[:, cl], start=False, stop=True)
                        nc.scalar.activation(nrm[:, cl], ss[:], AF.Sqrt)
                    inv = nrmp.tile([P, IBW], f32, tag="inv", name=f"inv_{t}{g}")
                    nc.vector.reciprocal_approx_fast(inv[:], nrm[:])
                    nc.gpsimd.tensor_mul(scl[t, 0][:, gl], raw[t, 0][:, gl], inv[:])
                    nc.gpsimd.tensor_mul(scl[t, 1][:, gl], raw[t, 1][:, gl], inv[:])

                # --- main GEMM: out[m*128:, :] = scl_cur[:, m].T @ scl_ref ---
                # emitted per half-m-tile (2 psum tiles, then a 512 KiB DMA
                # on the otherwise-idle sync ring) so PSUM drain never
                # couples the ACT queue to DVE completions, and so m0's
                # first half can run before ref h1 is normalized.
                with tc.tile_pool(name="outp", bufs=9) as outp:
                    obs = {}

                    def gemm_half(m, half):
                        msl = slice(m * P, (m + 1) * P)
                        if half == 0:
                            obs[m] = outp.tile([P, HW], f16, tag="ob", name=f"ob{m}")
                        ob = obs[m]
                        for q in (2 * half, 2 * half + 1):
                            pt = psp.tile(
                                [P, 2 * FD], f32, tag="pt", name=f"pt{m}_{q}", bufs=3
                            )
                            for sub in range(2):
                                n = q * 2 + sub
                                nsl = slice(n * FD, (n + 1) * FD)
                                psl = slice(sub * FD, (sub + 1) * FD)
                                nc.tensor.matmul(
                                    pt[:, psl], scl["cur", 0][:, msl],
                                    scl["ref", 0][:, nsl],
                                    start=True, stop=False,
                                )
                                nc.tensor.matmul(
                                    pt[:, psl], scl["cur", 1][:, msl],
                                    scl["ref", 1][:, nsl],
                                    start=False, stop=True,
                                )
                            osl = slice(q * 2 * FD, (q + 1) * 2 * FD)
                            # evac psum -> fp16 staging; ~44/56 ACT/DVE split
                            # (ACT also runs the squares/sqrt of the norms)
                            if q == 0 or (q == 2 and m % 4 != 3):
                                nc.scalar.activation(ob[:, osl], pt[:], AF.Copy)
                            else:
                                nc.vector.tensor_copy(ob[:, osl], pt[:])
                            if m == MT - 1:
                                # fine-grained drain for the last tile
                                nc.sync.dma_start(out_d[msl, osl], ob[:, osl])
                        if m < MT - 1:
                            hsl = slice(half * (HW // 2), (half + 1) * (HW // 2))
                            nc.sync.dma_start(out_d[msl, hsl], ob[:, hsl])

                    # ref g0 and cur g0 (columns 0-2047 = cur chunks 0-3)
                    # unblock the h0 halves of m0-m7, which keep the PE fed
                    # while ref/cur h1 are normalized; after the head no
                    # norm work remains in the steady-state loop.
                    norm_group("ref", 0)
                    norm_group("cur", 0)
                    for m in range(4):
                        gemm_half(m, 0)
                    norm_group("ref", 1)
                    for m in range(4, 8):
                        gemm_half(m, 0)
                    norm_group("cur", 1)
                    for m in range(8):
                        gemm_half(m, 1)
                    for m in range(8, MT):
                        gemm_half(m, 0)
                        gemm_half(m, 1)

    nc.compile()
    return nc


def _get_nc():
    global _cached_nc
    if _cached_nc is None:
        _cached_nc = _build()
    return _cached_nc


def _run(cur, ref, trace=False, **kw):
    """cur/ref: [B, C, HW] float. Returns (out [B, HW, HW] f32, results)."""
    nc = _get_nc()
    cur = np.ascontiguousarray(np.asarray(cur).astype(np.float16))
    ref = np.ascontiguousarray(np.asarray(ref).astype(np.float16))
    in_maps = [{"cur": cur[b], "ref": ref[b]} for b in range(B)]
    res = run_bass_kernel_spmd(nc, in_maps, list(range(B)), trace=trace, **kw)
    out = np.stack([res.results[b]["out"] for b in range(B)]).astype(np.float32)
    return out, res


def kernel(ref_features, cur_features):
    ref = np.asarray(ref_features, np.float32).reshape(B, C, HW)
    cur = np.asarray(cur_features, np.float32).reshape(B, C, HW)
    out, _ = _run(cur, ref)
    return out.reshape(B, H, W, H, W)


# revision 31
# speedup vs baseline: 1.0250x; 1.0250x over previous
"""CorrCosine TRN2 kernel.

out[b, i, j, h, w] = <cur[b,:,i,j]/||cur[b,:,i,j]||, ref[b,:,h,w]/||ref[b,:,h,w]||>

Data-parallel over batch B=8 across the 8 NeuronCores; per core one
[4096 x 256] @ [256 x 4096] GEMM plus the two L2 normalizations.

The original fp32 version was output-DMA-bound (64 MiB fp32 result
~190us of HBM writes per core).  This version runs the whole pipeline in
fp16: inputs are cast on the host, the GEMM runs fp16 x fp16 -> fp32
PSUM (full PE rate + fast weight load), the output is stored fp16 and
upcast on the host.  Norms are computed on-chip in fp32 via an all-ones
stationary matmul (which leaves the result broadcast across all 128
partitions), and BOTH operands are pre-scaled by 1/norm on the
otherwise-idle GpSimd engine, so PSUM evacuation is a plain copy split
between ACT and DVE.  All output DMA goes through the sync-engine HWDGE
ring, one 512 KiB transfer per half m-tile, so the ACT/DVE queues never
stall on each other's completions.
"""

import numpy as np

from concourse import bacc, mybir
from concourse import tile
from concourse.bass_utils import run_bass_kernel_spmd

B, C, H, W = 8, 256, 64, 64
HW = H * W            # 4096
P = 128               # partitions
KT = C // P           # 2 k-tiles
FD = 512              # psum bank free dim (fp32)
NCH = HW // FD        # 8 norm chunks
MT = HW // P          # 32 m-tiles
IBW = 2048            # input DMA width (512 KiB fp16 chunks)

f16 = mybir.dt.float16
f32 = mybir.dt.float32
AF = mybir.ActivationFunctionType

_cached_nc = None


def _build():
    nc = bacc.Bacc("TRN2", target_bir_lowering=False, debug=False)
    cur_d = nc.dram_tensor("cur", [C, HW], f16, kind="ExternalInput")
    ref_d = nc.dram_tensor("ref", [C, HW], f16, kind="ExternalInput")
    out_d = nc.dram_tensor("out", [HW, HW], f16, kind="ExternalOutput")

    with tile.TileContext(nc) as tc:
        with (
            tc.tile_pool(name="opnd", bufs=1) as opnd,
            tc.tile_pool(name="cst", bufs=1) as cstp,
            tc.tile_pool(name="ps", bufs=8, space="PSUM") as psp,
        ):
            ones_f = cstp.tile([P, P], f32, tag="ones_f", name="ones_f")
            nc.gpsimd.memset(ones_f[:], 1.0)
            ones = cstp.tile([P, P], f16, tag="ones", name="ones")
            nc.vector.tensor_copy(ones[:], ones_f[:])

            raw = {}
            scl = {}
            for t in ("ref", "cur"):
                for k in range(KT):
                    raw[t, k] = opnd.tile(
                        [P, HW], f16, tag=f"raw_{t}{k}", name=f"raw_{t}{k}"
                    )
                    scl[t, k] = opnd.tile(
                        [P, HW], f16, tag=f"scl_{t}{k}", name=f"scl_{t}{k}"
                    )

            # input order: ref h0 / cur h0 / cur h1 on the sync HWDGE ring
            # (~0.6us first-byte latency); ref h1 in parallel on the GpSimd
            # SWDGE ring so the second half of the ref normalization isn't
            # gated behind four queued sync-ring transfers.
            for i, t, eng in (
                (0, "ref", nc.sync),
                (1, "ref", nc.gpsimd),
                (0, "cur", nc.sync),
                (1, "cur", nc.sync),
            ):
                src = ref_d if t == "ref" else cur_d
                for k in range(KT):
                    eng.dma_start(
                        raw[t, k][:, i * IBW:(i + 1) * IBW],
                        src[k * P:(k + 1) * P, i * IBW:(i + 1) * IBW],
                    )

            with (
                tc.tile_pool(name="sq", bufs=3) as sqp,
                tc.tile_pool(name="nrm", bufs=3) as nrmp,
            ):
                # warmup matmuls while the inputs stream in: the PE pstate
                # needs ~3us of continuous execution to reach 2.4 GHz, so
                # spin it on dummy data during the otherwise-idle fill.
                warm = cstp.tile([P, FD], f16, tag="warm", name="warm")
                nc.gpsimd.memset(warm[:], 0.0)
                for w in range(12):
                    wt = psp.tile([P, FD], f32, tag="ss", name=f"warm{w}", bufs=2)
                    nc.tensor.matmul(wt[:], ones[:], warm[:], start=True, stop=True)
                def norm_scale(t, ch):
                    # 1/||.|| for columns [ch*FD, (ch+1)*FD) of tensor t,
                    # broadcast on all partitions; scale both k-tiles with it.
                    # fp16 squares keep the ones-matmul on the fast PE path
                    sl = slice(ch * FD, (ch + 1) * FD)
                    sq0 = sqp.tile([P, FD], f16, tag="sq", name=f"sq0_{t}{ch}")
                    sq1 = sqp.tile([P, FD], f16, tag="sq", name=f"sq1_{t}{ch}")
                    nc.scalar.activation(sq0[:], raw[t, 0][:, sl], AF.Square)
                    nc.scalar.activation(sq1[:], raw[t, 1][:, sl], AF.Square)
                    ss = psp.tile([P, FD], f32, tag="ss", name=f"ss_{t}{ch}", bufs=2)
                    nc.tensor.matmul(ss[:], ones[:], sq0[:], start=True, stop=False)
                    nc.tensor.matmul(ss[:], ones[:], sq1[:], start=False, stop=True)
                    nrm = nrmp.tile([P, FD], f32, tag="nrm", name=f"nrm_{t}{ch}")
                    nc.scalar.activation(nrm[:], ss[:], AF.Sqrt)
                    inv = nrmp.tile([P, FD], f32, tag="inv", name=f"inv_{t}{ch}")
                    nc.vector.reciprocal_approx_fast(inv[:], nrm[:])
                    nc.gpsimd.tensor_mul(scl[t, 0][:, sl], raw[t, 0][:, sl], inv[:])
                    nc.gpsimd.tensor_mul(scl[t, 1][:, sl], raw[t, 1][:, sl], inv[:])

                # --- main GEMM: out[m*128:, :] = scl_cur[:, m].T @ scl_ref ---
                # emitted per half-m-tile (2 psum tiles, then a 512 KiB DMA
                # on the otherwise-idle sync ring) so PSUM drain never
                # couples the ACT queue to DVE completions, and so m0's
                # first half can run before ref h1 is normalized.
                with tc.tile_pool(name="outp", bufs=4) as outp:
                    obs = {}

                    def gemm_half(m, half):
                        msl = slice(m * P, (m + 1) * P)
                        if half == 0:
                            obs[m] = outp.tile([P, HW], f16, tag="ob", name=f"ob{m}")
                        ob = obs[m]
                        for q in (2 * half, 2 * half + 1):
                            pt = psp.tile(
                                [P, 2 * FD], f32, tag="pt", name=f"pt{m}_{q}", bufs=3
                            )
                            for sub in range(2):
                                n = q * 2 + sub
                                nsl = slice(n * FD, (n + 1) * FD)
                                psl = slice(sub * FD, (sub + 1) * FD)
                                nc.tensor.matmul(
                                    pt[:, psl], scl["cur", 0][:, msl],
                                    scl["ref", 0][:, nsl],
                                    start=True, stop=False,
                                )
                                nc.tensor.matmul(
                                    pt[:, psl], scl["cur", 1][:, msl],
                                    scl["ref", 1][:, nsl],
                                    start=False, stop=True,
                                )
                            osl = slice(q * 2 * FD, (q + 1) * 2 * FD)
                            # evac psum -> fp16 staging; ~44/56 ACT/DVE split
                            # (ACT also runs the squares/sqrt of the norms)
                            if q == 0 or (q == 2 and m % 4 != 3):
                                nc.scalar.activation(ob[:, osl], pt[:], AF.Copy)
                            else:
                                nc.vector.tensor_copy(ob[:, osl], pt[:])
                            if m == MT - 1:
                                # fine-grained drain for the last tile
                                nc.sync.dma_start(out_d[msl, osl], ob[:, osl])
                        if m < MT - 1:
                            hsl = slice(half * (HW // 2), (half + 1) * (HW // 2))
                            nc.sync.dma_start(out_d[msl, hsl], ob[:, hsl])

                    # ref chunks 0-3 (h0) and cur chunk 0 unblock m0's first
                    # half; ref h1 + cur chunk 1/2 norms overlap with it.
                    for ch in range(NCH // 2):
                        norm_scale("ref", ch)
                    norm_scale("cur", 0)
                    gemm_half(0, 0)
                    for ch in range(NCH // 2, NCH):
                        norm_scale("ref", ch)
                    norm_scale("cur", 1)
                    gemm_half(0, 1)
                    norm_scale("cur", 2)

                    mpc = MT // NCH  # m-tiles per cur chunk (4)
                    for m in range(1, MT):
                        if m % mpc == 0 and m // mpc + 2 <= NCH - 1:
                            norm_scale("cur", m // mpc + 2)
                        gemm_half(m, 0)
                        gemm_half(m, 1)

    nc.compile()
    return nc


def _get_nc():
    global _cached_nc
    if _cached_nc is None:
        _cached_nc = _build()
    return _cached_nc


def _run(cur, ref, trace=False, **kw):
    """cur/ref: [B, C, HW] float. Returns (out [B, HW, HW] f32, results)."""
    nc = _get_nc()
    cur = np.ascontiguousarray(np.asarray(cur).astype(np.float16))
    ref = np.ascontiguousarray(np.asarray(ref).astype(np.float16))
    in_maps = [{"cur": cur[b], "ref": ref[b]} for b in range(B)]
    res = run_bass_kernel_spmd(nc, in_maps, list(range(B)), trace=trace, **kw)
    out = np.stack([res.results[b]["out"] for b in range(B)]).astype(np.float32)
    return out, res


def kernel(ref_features, cur_features):
    ref = np.asarray(ref_features, np.float32).reshape(B, C, HW)
    cur = np.asarray(cur_features, np.float32).reshape(B, C, HW)
    out, _ = _run(cur, ref)
    return out.reshape(B, H, W, H, W)


# revision 32
# speedup vs baseline: 1.0265x; 1.0015x over previous
"""CorrCosine TRN2 kernel.

out[b, i, j, h, w] = <cur[b,:,i,j]/||cur[b,:,i,j]||, ref[b,:,h,w]/||ref[b,:,h,w]||>

Data-parallel over batch B=8 across the 8 NeuronCores; per core one
[4096 x 256] @ [256 x 4096] GEMM plus the two L2 normalizations.

The original fp32 version was output-DMA-bound (64 MiB fp32 result
~190us of HBM writes per core).  This version runs the whole pipeline in
fp16: inputs are cast on the host, the GEMM runs fp16 x fp16 -> fp32
PSUM (full PE rate + fast weight load), the output is stored fp16 and
upcast on the host.  Norms are computed on-chip in fp32 via an all-ones
stationary matmul (which leaves the result broadcast across all 128
partitions), and BOTH operands are pre-scaled by 1/norm on the
otherwise-idle GpSimd engine, so PSUM evacuation is a plain copy split
between ACT and DVE.  All output DMA goes through the sync-engine HWDGE
ring, one 512 KiB transfer per half m-tile, so the ACT/DVE queues never
stall on each other's completions.
"""

import numpy as np

from concourse import bacc, mybir
from concourse import tile
from concourse.bass_utils import run_bass_kernel_spmd

B, C, H, W = 8, 256, 64, 64
HW = H * W            # 4096
P = 128               # partitions
KT = C // P           # 2 k-tiles
FD = 512              # psum bank free dim (fp32)
NCH = HW // FD        # 8 norm chunks
MT = HW // P          # 32 m-tiles
IBW = 2048            # input DMA width (512 KiB fp16 chunks)

f16 = mybir.dt.float16
f32 = mybir.dt.float32
AF = mybir.ActivationFunctionType

_cached_nc = None


def _build():
    nc = bacc.Bacc("TRN2", target_bir_lowering=False, debug=False)
    cur_d = nc.dram_tensor("cur", [C, HW], f16, kind="ExternalInput")
    ref_d = nc.dram_tensor("ref", [C, HW], f16, kind="ExternalInput")
    out_d = nc.dram_tensor("out", [HW, HW], f16, kind="ExternalOutput")

    with tile.TileContext(nc) as tc:
        with (
            tc.tile_pool(name="opnd", bufs=1) as opnd,
            tc.tile_pool(name="cst", bufs=1) as cstp,
            tc.tile_pool(name="ps", bufs=8, space="PSUM") as psp,
        ):
            ones_f = cstp.tile([P, P], f32, tag="ones_f", name="ones_f")
            nc.gpsimd.memset(ones_f[:], 1.0)
            ones = cstp.tile([P, P], f16, tag="ones", name="ones")
            nc.vector.tensor_copy(ones[:], ones_f[:])

            raw = {}
            scl = {}
            for t in ("ref", "cur"):
                for k in range(KT):
                    raw[t, k] = opnd.tile(
                        [P, HW], f16, tag=f"raw_{t}{k}", name=f"raw_{t}{k}"
                    )
                    scl[t, k] = opnd.tile(
                        [P, HW], f16, tag=f"scl_{t}{k}", name=f"scl_{t}{k}"
                    )

            # input order: ref h0 / cur h0 / cur h1 on the sync HWDGE ring
            # (~0.6us first-byte latency); ref h1 in parallel on the GpSimd
            # SWDGE ring so the second half of the ref normalization isn't
            # gated behind four queued sync-ring transfers.
            for i, t, eng in (
                (0, "ref", nc.sync),
                (1, "ref", nc.gpsimd),
                (0, "cur", nc.sync),
                (1, "cur", nc.sync),
            ):
                src = ref_d if t == "ref" else cur_d
                for k in range(KT):
                    eng.dma_start(
                        raw[t, k][:, i * IBW:(i + 1) * IBW],
                        src[k * P:(k + 1) * P, i * IBW:(i + 1) * IBW],
                    )

            with (
                tc.tile_pool(name="sq", bufs=3) as sqp,
                tc.tile_pool(name="nrm", bufs=3) as nrmp,
            ):
                def norm_scale(t, ch):
                    # 1/||.|| for columns [ch*FD, (ch+1)*FD) of tensor t,
                    # broadcast on all partitions; scale both k-tiles with it.
                    # fp16 squares keep the ones-matmul on the fast PE path
                    sl = slice(ch * FD, (ch + 1) * FD)
                    sq0 = sqp.tile([P, FD], f16, tag="sq", name=f"sq0_{t}{ch}")
                    sq1 = sqp.tile([P, FD], f16, tag="sq", name=f"sq1_{t}{ch}")
                    nc.scalar.activation(sq0[:], raw[t, 0][:, sl], AF.Square)
                    nc.scalar.activation(sq1[:], raw[t, 1][:, sl], AF.Square)
                    ss = psp.tile([P, FD], f32, tag="ss", name=f"ss_{t}{ch}", bufs=2)
                    nc.tensor.matmul(ss[:], ones[:], sq0[:], start=True, stop=False)
                    nc.tensor.matmul(ss[:], ones[:], sq1[:], start=False, stop=True)
                    nrm = nrmp.tile([P, FD], f32, tag="nrm", name=f"nrm_{t}{ch}")
                    nc.scalar.activation(nrm[:], ss[:], AF.Sqrt)
                    inv = nrmp.tile([P, FD], f32, tag="inv", name=f"inv_{t}{ch}")
                    nc.vector.reciprocal_approx_fast(inv[:], nrm[:])
                    nc.gpsimd.tensor_mul(scl[t, 0][:, sl], raw[t, 0][:, sl], inv[:])
                    nc.gpsimd.tensor_mul(scl[t, 1][:, sl], raw[t, 1][:, sl], inv[:])

                # --- main GEMM: out[m*128:, :] = scl_cur[:, m].T @ scl_ref ---
                # emitted per half-m-tile (2 psum tiles, then a 512 KiB DMA
                # on the otherwise-idle sync ring) so PSUM drain never
                # couples the ACT queue to DVE completions, and so m0's
                # first half can run before ref h1 is normalized.
                with tc.tile_pool(name="outp", bufs=4) as outp:
                    obs = {}

                    def gemm_half(m, half):
                        msl = slice(m * P, (m + 1) * P)
                        if half == 0:
                            obs[m] = outp.tile([P, HW], f16, tag="ob", name=f"ob{m}")
                        ob = obs[m]
                        for q in (2 * half, 2 * half + 1):
                            pt = psp.tile(
                                [P, 2 * FD], f32, tag="pt", name=f"pt{m}_{q}", bufs=3
                            )
                            for sub in range(2):
                                n = q * 2 + sub
                                nsl = slice(n * FD, (n + 1) * FD)
                                psl = slice(sub * FD, (sub + 1) * FD)
                                nc.tensor.matmul(
                                    pt[:, psl], scl["cur", 0][:, msl],
                                    scl["ref", 0][:, nsl],
                                    start=True, stop=False,
                                )
                                nc.tensor.matmul(
                                    pt[:, psl], scl["cur", 1][:, msl],
                                    scl["ref", 1][:, nsl],
                                    start=False, stop=True,
                                )
                            osl = slice(q * 2 * FD, (q + 1) * 2 * FD)
                            # evac psum -> fp16 staging; ~44/56 ACT/DVE split
                            # (ACT also runs the squares/sqrt of the norms)
                            if q == 0 or (q == 2 and m % 4 != 3):
                                nc.scalar.activation(ob[:, osl], pt[:], AF.Copy)
                            else:
                                nc.vector.tensor_copy(ob[:, osl], pt[:])
                            if m == MT - 1:
                                # fine-grained drain for the last tile
                                nc.sync.dma_start(out_d[msl, osl], ob[:, osl])
                        if m < MT - 1:
                            hsl = slice(half * (HW // 2), (half + 1) * (HW // 2))
                            nc.sync.dma_start(out_d[msl, hsl], ob[:, hsl])

                    # ref chunks 0-3 (h0) and cur chunk 0 unblock m0's first
                    # half; ref h1 + cur chunk 1/2 norms overlap with it.
                    for ch in range(NCH // 2):
                        norm_scale("ref", ch)
                    norm_scale("cur", 0)
                    gemm_half(0, 0)
                    for ch in range(NCH // 2, NCH):
                        norm_scale("ref", ch)
                    norm_scale("cur", 1)
                    gemm_half(0, 1)
                    norm_scale("cur", 2)

                    mpc = MT // NCH  # m-tiles per cur chunk (4)
                    for m in range(1, MT):
                        if m % mpc == 0 and m // mpc + 2 <= NCH - 1:
                            norm_scale("cur", m // mpc + 2)
                        gemm_half(m, 0)
                        gemm_half(m, 1)

    nc.compile()
    return nc


def _get_nc():
    global _cached_nc
    if _cached_nc is None:
        _cached_nc = _build()
    return _cached_nc


def _run(cur, ref, trace=False, **kw):
    """cur/ref: [B, C, HW] float. Returns (out [B, HW, HW] f32, results)."""
    nc = _get_nc()
    cur = np.ascontiguousarray(np.asarray(cur).astype(np.float16))
    ref = np.ascontiguousarray(np.asarray(ref).astype(np.float16))
    in_maps = [{"cur": cur[b], "ref": ref[b]} for b in range(B)]
    res = run_bass_kernel_spmd(nc, in_maps, list(range(B)), trace=trace, **kw)
    out = np.stack([res.results[b]["out"] for b in range(B)]).astype(np.float32)
    return out, res


def kernel(ref_features, cur_features):
    ref = np.asarray(ref_features, np.float32).reshape(B, C, HW)
    cur = np.asarray(cur_features, np.float32).reshape(B, C, HW)
    out, _ = _run(cur, ref)
    return out.reshape(B, H, W, H, W)
